# Initial kernel scaffold
#
"""Fused conv1x1-attention-FFN kernel for 8 trn2 NeuronCores.

Reference computation (per batch b of 4, N = 64*64 = 4096 pixels, C = 256):
    q = Wq @ x_q + bq ; k = Wk @ x_kv + bk ; v = Wv @ x_kv + bv      [C, N]
    attn = softmax_over_keys(q^T k)                                   [N, N]
    av = v @ attn^T                                                   [C, N]
    out = W2 @ relu(W1 @ av + b1) + b2                                [C, N]

Sharding: 8 cores = 4 batches x 2 query-row halves. Each core computes the
full K/V for its batch (cheap vs the 4.3 GMAC attention) and attends its
2048 query rows against all 4096 keys - no collectives needed.

Host-side algebra (free preprocessing in _make_in_maps, done in float64):
    softmax over keys is invariant to per-query offsets, so
        scores ~ xq^T (Wq^T Wk) xkv + (Wk^T bq)^T xkv
    A  = Wk^T Wq          -> k' contracted from xkv on device (q proj gone);
    u  = Wk^T bq          -> per-KEY bias t[m] = u . xkv_m, computed as a
                             257th streamed column of the value projection
                             and applied as the exp() per-partition bias;
    relu(W1(av r) + b1) = relu((W1 av) r + b1)  (r = 1/sum > 0), so
    F  = (W1 Wv)^T, bv' = W1 bv -> the value projection directly produces
                             v' = W1 v and the FFN hidden matmuls vanish.

On-chip layout (matmuls contract over the partition dim): scores are
TRANSPOSED, S^T[m, n] = sum_c k'[c,m] xq[c,n], so av[c,n] needs no on-chip
transpose. Per 512-query chunk: 32 key tiles, each = 2 score matmuls +
exp (ACT, bf16 out) + 2 av matmuls; softmax denominators accumulate on the
DVE (4 independent bf16 sub-accumulators) and reduce via 4 accumulating
ones-matmuls; 1/sum via DVE reciprocal; normalization by a broadcast
matmul + DVE muls; then relu (ACT) and the W2 matmuls. The m-loop is
explicitly software-pipelined 2 deep (scores for mi+2 and exp for mi+1
are emitted ahead of mi's av matmuls) so the PE never waits on the
PSUM->exp->SBUF round trip. Chunk j's FFN is emitted in staged pieces
during chunk j+1's m-loop.

Inputs ship as bf16 (xq, xkv, A, Fu) to halve the startup HBM traffic;
W2/biases stay f32 (f32r on device). PSUM accumulation is always fp32.
Measured end-to-end relative error ~2e-3 vs the fp64 reference (gate 2e-2).
"""
import sys

sys.path.insert(0, "/opt/trn_rl_repo")

import numpy as np
from concourse import bass, bacc, mybir, tile
from concourse.bass_utils import run_bass_kernel_spmd

F32 = mybir.dt.float32
CDT = mybir.dt.float32r  # f32r view of f32 weights for PE operands
BF16 = mybir.dt.bfloat16

B, C, H, W = 4, 256, 64, 64
N = H * W              # 4096 keys per batch
NL = N // 2            # 2048 query rows per core
CT = C // 128          # 2 channel tiles
MT = N // 128          # 32 key tiles
NCH = 512              # query-column chunk
NJ = NL // NCH         # 4 chunks
CV = C + 1             # value-projection columns: 256 channels + t-bias
WPK = C + CV + C + 2   # bf16 pack: A | Fu | W2^T | b1 | b2
AF = mybir.ActivationFunctionType


def _build():
    nc = bacc.Bacc(None, target_bir_lowering=False, debug=False)

    xq_d = nc.declare_dram_parameter("xq", [128, CT, NL], BF16, isOutput=False)
    xkv_d = nc.declare_dram_parameter("xkv", [128, CT, N], BF16, isOutput=False)
    wp_d = nc.declare_dram_parameter("wpack", [128, CT, WPK], BF16, isOutput=False)
    bv_d = nc.declare_dram_parameter("bvb", [128, CV], F32, isOutput=False)
    out_d = nc.declare_dram_parameter("out", [128, CT, NL], F32, isOutput=True)

    with tile.TileContext(nc) as tc:
        with (
            tc.tile_pool(name="const", bufs=1) as cpool,
            tc.tile_pool(name="big", bufs=1) as bpool,
            tc.tile_pool(name="work", bufs=2) as wpool,
            tc.tile_pool(name="et", bufs=4) as epool,
            tc.tile_pool(name="psum", bufs=1, space="PSUM") as pp,
        ):
            # ---- weights: one bf16 pack, sliced in place ----
            wp = cpool.tile([128, CT, WPK], BF16, tag="wp")
            bv_s = cpool.tile([128, CV], F32, tag="bv_s")

            def wa(ci, osl):  # score matrix A block
                return wp[:, ci, osl.start:osl.stop]

            def wfu(ci):      # value-proj streamed weights [c, 257]
                return wp[:, ci, C:C + CV]

            def w2sl(ci, osl):  # W2^T block
                return wp[:, ci, C + CV + osl.start:C + CV + osl.stop]

            def b1sl(ct):
                return wp[:, ct, C + CV + C:C + CV + C + 1]

            def b2sl(ct):
                return wp[:, ct, C + CV + C + 1:C + CV + C + 2]

            bf32 = cpool.tile([128, CT, 2], F32, tag="bf32")
            ones_f = cpool.tile([128, 1], F32, tag="ones_f")
            nc.vector.memset(ones_f[:], 1.0)
            ones_b = cpool.tile([128, 1], BF16, tag="ones_b")
            nc.vector.tensor_copy(ones_b[:], ones_f[:])
            onesrow_f = cpool.tile([1, 128], F32, tag="onesrow_f")
            nc.vector.memset(onesrow_f[:], 1.0)
            onesrow = cpool.tile([1, 128], CDT, tag="onesrow")
            nc.vector.tensor_copy(onesrow[:], onesrow_f[:])

            # ---- inputs: DMA issue order == stripe priority: the first
            # projections' and first attention chunk's deps lead the line ----
            xkv_r = bpool.tile([128, CT, N], BF16, tag="xkv_r")
            xq_r = bpool.tile([128, CT, NL], BF16, tag="xq_r")
            k_r = bpool.tile([128, CT, N], BF16, tag="k_r")
            vt_r = bpool.tile([128, MT, CV], BF16, tag="vt_r")

            def dkv(p):
                nc.sync.dma_start(xkv_r[:, :, p * 512:(p + 1) * 512],
                                  xkv_d[:, :, p * 512:(p + 1) * 512])

            def dq(p):
                nc.sync.dma_start(xq_r[:, :, p * 512:(p + 1) * 512],
                                  xq_d[:, :, p * 512:(p + 1) * 512])

            nc.sync.dma_start(wp[:, :, 0:C], wp_d[:, :, 0:C])
            dkv(0)
            nc.sync.dma_start(wp[:, :, C:WPK], wp_d[:, :, C:WPK])
            nc.sync.dma_start(bv_s[:], bv_d[:])
            dkv(1)
            dq(0)
            dkv(2)
            dq(1)
            for p in (3, 4, 5):
                dkv(p)
            dq(2)
            dkv(6)
            dq(3)
            dkv(7)
            nc.vector.tensor_copy(bf32[:], wp[:, :, C + CV + C:C + CV + C + 2])

            def kproj(j):  # k' = A-contracted xkv, 512-col chunk j
                sl = slice(j * NCH, (j + 1) * NCH)
                for ct in range(CT):
                    ps = pp.tile([128, NCH], F32, tag="st", name=f"pk{j}_{ct}", bufs=3)
                    for ci in range(CT):
                        nc.tensor.matmul(ps[:], wa(ci, slice(ct * 128, ct * 128 + 128)),
                                         xkv_r[:, ci, sl],
                                         start=(ci == 0), stop=(ci == CT - 1))
                    nc.scalar.activation(k_r[:, ct, sl], ps[:], AF.Identity)

            def vtproj(mi):  # v'^T tile [keys, 256 ch + 1 t-bias col]
                ps = pp.tile([128, CV], F32, tag="st", name=f"pv{mi}", bufs=3)
                for ci in range(CT):
                    nc.tensor.matmul(ps[:], xkv_r[:, ci, mi * 128:(mi + 1) * 128],
                                     wfu(ci),
                                     start=(ci == 0), stop=(ci == CT - 1))
                nc.vector.tensor_add(vt_r[:, mi, :], ps[:], bv_s[:])

            for c in range(4):
                kproj(2 * c)
                kproj(2 * c + 1)
                for mi in range(8 * c, 8 * c + 8):
                    vtproj(mi)

            # ---- attention; chunk j's FFN runs during chunk j+1's m-loop ----
            ffn_state = {}

            def ffn_stages(j, last=False):
                """Yield (mi_trigger, emit_fn) pieces for chunk j's FFN."""
                sl = slice(j * NCH, (j + 1) * NCH)
                # Tail chunk: reuse the dead av0/av1 bank slots so the
                # out-matmul pair can overlap without extra PSUM banks.
                if last:
                    ftags = {"rbp": ("ffn", 1), "op0": ("av0", 2),
                             "op1": ("av1", 2)}
                else:
                    ftags = {k: ("ffn", 1) for k in ("rbp", "op0", "op1")}
                st_ = {}

                def s_recip():
                    av0, av1, smp = ffn_state.pop(j)
                    st_["av"] = (av0, av1)
                    r = wpool.tile([1, NCH], CDT, tag="recip", name=f"recip{j}",
                                   bufs=1)
                    with nc.allow_low_precision(reason="f32r reciprocal ~2^-13"):
                        nc.vector.reciprocal(r[:], smp[:])
                    st_["recip"] = r

                def s_rbp():
                    t, b = ftags["rbp"]
                    rbp = pp.tile([128, NCH], F32, tag=t, name=f"rbp{j}", bufs=b)
                    nc.tensor.matmul(rbp[:], onesrow[:], st_["recip"][:],
                                     start=True, stop=True)
                    st_["rbp"] = rbp
                    if last:
                        warm2 = pp.tile([128, NCH], F32, tag="st", name="warm2",
                                        bufs=3)
                        for _ in range(10):
                            nc.tensor.matmul(warm2[:], onesrow[:],
                                             st_["recip"][:], start=True,
                                             stop=True)
                        wrd2 = wpool.tile([1, 1], F32, tag="wrd", bufs=1)
                        nc.vector.tensor_copy(wrd2[:], warm2[0:1, 0:1])

                def s_avn():
                    rb = wpool.tile([128, NCH], F32, tag="rb", name=f"rb{j}", bufs=1)
                    nc.vector.tensor_copy(rb[:], st_["rbp"][:])
                    avn = wpool.tile([128, CT, NCH], CDT, tag="avn", name=f"avn{j}",
                                     bufs=1)
                    av0, av1 = st_["av"]
                    nc.vector.tensor_mul(avn[:, 0, :], av0[:], rb[:])
                    nc.vector.tensor_mul(avn[:, 1, :], av1[:], rb[:])
                    st_["avn"] = avn
                    st_["hid"] = wpool.tile([128, CT, NCH], BF16, tag="hid",
                                            name=f"hid{j}", bufs=1)
                    st_["outp"] = wpool.tile([128, CT, NCH], F32, tag="outp",
                                             name=f"outp{j}", bufs=1)

                def s_relu(ot):
                    def go():
                        nc.scalar.activation(st_["hid"][:, ot, :],
                                             st_["avn"][:, ot, :], AF.Relu,
                                             bias=b1sl(ot))
                    return go

                def s_out(ot):
                    def go():
                        t, b = ftags[f"op{ot}"]
                        op = pp.tile([128, NCH], F32, tag=t,
                                     name=f"op{j}_{ot}", bufs=b)
                        for ci in range(CT):
                            nc.tensor.matmul(
                                op[:], w2sl(ci, slice(ot * 128, ot * 128 + 128)),
                                st_["hid"][:, ci, :], start=(ci == 0), stop=(ci == CT - 1))
                        if ot == 0:
                            nc.scalar.activation(st_["outp"][:, ot, :], op[:],
                                                 AF.Identity, bias=b2sl(ot))
                        else:
                            nc.vector.tensor_scalar_add(st_["outp"][:, ot, :],
                                                        op[:], bf32[:, ot, 1:2])
                    return go

                def s_dma(ot):
                    def go():
                        nc.sync.dma_start(out_d[:, ot, sl], st_["outp"][:, ot, :])
                    return go

                return [(1, s_recip), (5, s_rbp), (7, s_avn),
                        (9, s_relu(0)), (11, s_relu(1)),
                        (14, s_out(0)), (16, s_dma(0)),
                        (17, s_out(1)), (20, s_dma(1))]

            for j in range(NJ):
                sl = slice(j * NCH, (j + 1) * NCH)
                av0 = pp.tile([128, NCH], F32, tag="av0", name=f"av0_{j}", bufs=2)
                av1 = pp.tile([128, NCH], F32, tag="av1", name=f"av1_{j}", bufs=2)
                acc = wpool.tile([128, 4, NCH], BF16, tag="acc", name=f"acc{j}",
                                 bufs=2)
                pending = ffn_stages(j - 1) if j > 0 else []

                def sp_mm(mi):
                    sp = pp.tile([128, NCH], F32, tag="st", name=f"sp{j}_{mi}",
                                 bufs=3)
                    for ci in range(CT):
                        nc.tensor.matmul(sp[:], k_r[:, ci, mi * 128:(mi + 1) * 128],
                                         xq_r[:, ci, sl], start=(ci == 0),
                                         stop=(ci == CT - 1))
                    return sp

                def exp_mm(mi, sp):
                    et = epool.tile([128, NCH], BF16, tag="et", name=f"et{j}_{mi}")
                    nc.scalar.activation(et[:], sp[:], AF.Exp,
                                         bias=vt_r[:, mi, C:C + 1])
                    return et

                # 2-deep software pipeline: scores for mi+2 and exp for mi+1
                # are emitted (= prioritized) ahead of mi's AV matmuls, so the
                # PE never waits on the PSUM->exp->SBUF round trip.
                sps = {0: sp_mm(0), 1: sp_mm(1)}
                ets = {0: exp_mm(0, sps.pop(0))}
                for mi in range(MT):
                    if mi + 2 < MT:
                        sps[mi + 2] = sp_mm(mi + 2)
                    if mi + 1 < MT:
                        ets[mi + 1] = exp_mm(mi + 1, sps.pop(mi + 1))
                    et = ets.pop(mi)
                    first, last = mi == 0, mi == MT - 1
                    nc.tensor.matmul(av0[:], vt_r[:, mi, 0:128], et[:], start=first, stop=last)
                    nc.tensor.matmul(av1[:], vt_r[:, mi, 128:256], et[:], start=first, stop=last)
                    if j == NJ - 1 and mi == MT - 1:
                        last_et = et  # summed by a direct matmul, not the chain
                    else:
                        g = mi // 8
                        if mi % 8 == 0:
                            nc.vector.tensor_copy(acc[:, g, :], et[:])
                        else:
                            nc.vector.tensor_add(acc[:, g, :], acc[:, g, :], et[:])
                    while pending and pending[0][0] == mi:
                        pending.pop(0)[1]()
                # softmax denominators: accumulating partition-reduce matmuls
                lastj = j == NJ - 1
                smp = pp.tile([1, NCH], F32, tag="st" if lastj else "ffn",
                              name=f"smp{j}", bufs=3 if lastj else 1)
                for g in range(4):
                    nc.tensor.matmul(smp[:], ones_b[:], acc[:, g, :],
                                     start=(g == 0), stop=(g == 3 and not lastj))
                if lastj:
                    nc.tensor.matmul(smp[:], ones_b[:], last_et[:],
                                     start=False, stop=True)
                    warm = pp.tile([1, NCH], F32, tag="st", name="warm", bufs=3)
                    for _ in range(14):
                        nc.tensor.matmul(warm[:], ones_b[:], last_et[:],
                                         start=True, stop=True)
                    wrd = wpool.tile([1, 1], F32, tag="wrd", bufs=1)
                    nc.vector.tensor_copy(wrd[:], warm[:, 0:1])
                ffn_state[j] = (av0, av1, smp)
            for _, fn in ffn_stages(NJ - 1, last=True):
                fn()
    nc.compile()
    return nc


_NC_CACHE = None


def _get_nc():
    global _NC_CACHE
    if _NC_CACHE is None:
        _NC_CACHE = _build()
    return _NC_CACHE


def _fold(a, dt=np.float32):
    """[C, X] -> [128, CT, X] with channel tile as middle dim, contiguous."""
    x = np.asarray(a, dtype=dt)
    return np.ascontiguousarray(x.reshape(CT, 128, -1).transpose(1, 0, 2))


def _make_in_maps(inputs):
    import ml_dtypes

    bf16 = ml_dtypes.bfloat16
    f = {k: np.asarray(v, np.float64) for k, v in inputs.items()}
    query_input = f["query_input"].reshape(B, C, N)
    key_value_input = f["key_value_input"].reshape(B, C, N)

    # Host-side algebra (see module docstring): softmax-invariant rewrite of
    # the score bilinear form, and the W1-fold through the value projection.
    A = f["Wk"].T @ f["Wq"]                      # [Cin, Cin]
    u = f["Wk"].T @ f["bq"]                      # [Cin]
    Fu = np.concatenate([(f["W1"] @ f["Wv"]).T, u[:, None]], axis=1)  # [C, CV]
    wpack = np.concatenate(
        [A, Fu, f["W2"].T, f["b1"][:, None], f["b2"][:, None]], axis=1)
    bvp = np.concatenate([f["W1"] @ f["bv"], [0.0]])                  # [CV]

    base = {
        "wpack": _fold(wpack, bf16),
        "bvb": np.ascontiguousarray(
            np.broadcast_to(bvp.astype(np.float32)[None, :], (128, CV))),
    }
    in_maps = []
    for core in range(8):
        b, h = divmod(core, 2)
        m = dict(base)
        m["xq"] = _fold(query_input[b][:, h * NL:(h + 1) * NL], bf16)
        m["xkv"] = _fold(key_value_input[b], bf16)
        in_maps.append(m)
    return in_maps


def kernel(query_input, key_value_input, Wq, bq, Wk, bk, Wv, bv, W1, b1, W2, b2):
    in_maps = _make_in_maps(dict(
        query_input=query_input, key_value_input=key_value_input,
        Wq=Wq, bq=bq, Wk=Wk, bk=bk, Wv=Wv, bv=bv, W1=W1, b1=b1, W2=W2, b2=b2))
    nc = _get_nc()
    res = run_bass_kernel_spmd(nc, in_maps, core_ids=list(range(8)))

    out = np.empty((B, C, N), dtype=np.float32)
    for core in range(8):
        b, h = divmod(core, 2)
        o = res.results[core]["out"]  # [128, CT, NL]
        out[b][:, h * NL:(h + 1) * NL] = o.transpose(1, 0, 2).reshape(C, NL)
    return out.reshape(B, C, H, W)



# revision 1
# speedup vs baseline: 1.0056x; 1.0056x over previous
"""Fused conv1x1-attention-FFN kernel for 8 trn2 NeuronCores.

Reference computation (per batch b of 4, N = 64*64 = 4096 pixels, C = 256):
    q = Wq @ x_q + bq ; k = Wk @ x_kv + bk ; v = Wv @ x_kv + bv      [C, N]
    attn = softmax_over_keys(q^T k)                                   [N, N]
    av = v @ attn^T                                                   [C, N]
    out = W2 @ relu(W1 @ av + b1) + b2                                [C, N]

Sharding: 8 cores = 4 batches x 2 query-row halves. Each core computes the
full K/V for its batch (cheap vs the 4.3 GMAC attention) and attends its
2048 query rows against all 4096 keys - no collectives needed.

Host-side algebra (free preprocessing in _make_in_maps, done in float64):
    softmax over keys is invariant to per-query offsets, so
        scores ~ xq^T (Wq^T Wk) xkv + (Wk^T bq)^T xkv
    A  = Wk^T Wq          -> k' contracted from xkv on device (q proj gone);
    u  = Wk^T bq          -> per-KEY bias t[m] = u . xkv_m, computed as a
                             257th streamed column of the value projection
                             and applied as the exp() per-partition bias;
    relu(W1(av r) + b1) = relu((W1 av) r + b1)  (r = 1/sum > 0), so
    F  = (W1 Wv)^T, bv' = W1 bv -> the value projection directly produces
                             v' = W1 v and the FFN hidden matmuls vanish.

On-chip layout (matmuls contract over the partition dim): scores are
TRANSPOSED, S^T[m, n] = sum_c k'[c,m] xq[c,n], so av[c,n] needs no on-chip
transpose. Per 512-query chunk: 32 key tiles, each = 2 score matmuls +
exp (ACT, bf16 out) + 2 av matmuls; softmax denominators accumulate on the
DVE (4 independent bf16 sub-accumulators) and reduce via 4 accumulating
ones-matmuls; 1/sum via DVE reciprocal; normalization by a broadcast
matmul + DVE muls; then relu (ACT) and the W2 matmuls. The m-loop is
explicitly software-pipelined 2 deep (scores for mi+2 and exp for mi+1
are emitted ahead of mi's av matmuls) so the PE never waits on the
PSUM->exp->SBUF round trip. Chunk j's FFN is emitted in staged pieces
during chunk j+1's m-loop.

Inputs ship as bf16 (xq, xkv, A, Fu) to halve the startup HBM traffic;
W2/biases stay f32 (f32r on device). PSUM accumulation is always fp32.
Measured end-to-end relative error ~2e-3 vs the fp64 reference (gate 2e-2).
"""
import sys

sys.path.insert(0, "/opt/trn_rl_repo")

import numpy as np
from concourse import bass, bacc, mybir, tile
from concourse.bass_utils import run_bass_kernel_spmd

F32 = mybir.dt.float32
CDT = mybir.dt.float32r  # f32r view of f32 weights for PE operands
BF16 = mybir.dt.bfloat16

B, C, H, W = 4, 256, 64, 64
N = H * W              # 4096 keys per batch
NL = N // 2            # 2048 query rows per core
CT = C // 128          # 2 channel tiles
MT = N // 128          # 32 key tiles
NCH = 512              # query-column chunk
NJ = NL // NCH         # 4 chunks
CV = C + 1             # value-projection columns: 256 channels + t-bias
WPK = C + CV + C + 2   # bf16 pack: A | Fu | W2^T | b1 | b2
AF = mybir.ActivationFunctionType


def _build():
    nc = bacc.Bacc(None, target_bir_lowering=False, debug=False)

    xq_d = nc.declare_dram_parameter("xq", [128, CT, NL], BF16, isOutput=False)
    xkv_d = nc.declare_dram_parameter("xkv", [128, CT, N], BF16, isOutput=False)
    wp_d = nc.declare_dram_parameter("wpack", [128, CT, WPK], BF16, isOutput=False)
    bv_d = nc.declare_dram_parameter("bvb", [128, CV], F32, isOutput=False)
    out_d = nc.declare_dram_parameter("out", [128, CT, NL], F32, isOutput=True)

    with tile.TileContext(nc) as tc:
        with (
            tc.tile_pool(name="const", bufs=1) as cpool,
            tc.tile_pool(name="big", bufs=1) as bpool,
            tc.tile_pool(name="work", bufs=2) as wpool,
            tc.tile_pool(name="et", bufs=4) as epool,
            tc.tile_pool(name="psum", bufs=1, space="PSUM") as pp,
        ):
            # ---- weights: one bf16 pack, sliced in place ----
            wp = cpool.tile([128, CT, WPK], BF16, tag="wp")
            bv_s = cpool.tile([128, CV], F32, tag="bv_s")

            def wa(ci, osl):  # score matrix A block
                return wp[:, ci, osl.start:osl.stop]

            def wfu(ci):      # value-proj streamed weights [c, 257]
                return wp[:, ci, C:C + CV]

            def w2sl(ci, osl):  # W2^T block
                return wp[:, ci, C + CV + osl.start:C + CV + osl.stop]

            def b1sl(ct):
                return wp[:, ct, C + CV + C:C + CV + C + 1]

            def b2sl(ct):
                return wp[:, ct, C + CV + C + 1:C + CV + C + 2]

            bf32 = cpool.tile([128, CT, 2], F32, tag="bf32")
            ones_f = cpool.tile([128, 1], F32, tag="ones_f")
            nc.vector.memset(ones_f[:], 1.0)
            ones_b = cpool.tile([128, 1], BF16, tag="ones_b")
            nc.vector.tensor_copy(ones_b[:], ones_f[:])
            onesrow_f = cpool.tile([1, 128], F32, tag="onesrow_f")
            nc.vector.memset(onesrow_f[:], 1.0)
            onesrow = cpool.tile([1, 128], CDT, tag="onesrow")
            nc.vector.tensor_copy(onesrow[:], onesrow_f[:])

            # ---- inputs: DMA issue order == stripe priority: the first
            # projections' and first attention chunk's deps lead the line ----
            xkv_r = bpool.tile([128, CT, N], BF16, tag="xkv_r")
            xq_r = bpool.tile([128, CT, NL], BF16, tag="xq_r")
            k_r = bpool.tile([128, CT, N], BF16, tag="k_r")
            vt_r = bpool.tile([128, MT, CV], BF16, tag="vt_r")

            def dkv(p):
                nc.sync.dma_start(xkv_r[:, :, p * 512:(p + 1) * 512],
                                  xkv_d[:, :, p * 512:(p + 1) * 512])

            def dq(p):
                nc.sync.dma_start(xq_r[:, :, p * 512:(p + 1) * 512],
                                  xq_d[:, :, p * 512:(p + 1) * 512])

            nc.sync.dma_start(wp[:, :, 0:C], wp_d[:, :, 0:C])
            dkv(0)
            nc.sync.dma_start(wp[:, :, C:WPK], wp_d[:, :, C:WPK])
            nc.sync.dma_start(bv_s[:], bv_d[:])
            dkv(1)
            dq(0)
            dkv(2)
            dq(1)
            for p in (3, 4, 5):
                dkv(p)
            dq(2)
            dkv(6)
            dq(3)
            dkv(7)
            nc.vector.tensor_copy(bf32[:], wp[:, :, C + CV + C:C + CV + C + 2])

            def kproj(j):  # k' = A-contracted xkv, 512-col chunk j
                sl = slice(j * NCH, (j + 1) * NCH)
                for ct in range(CT):
                    ps = pp.tile([128, NCH], F32, tag="st", name=f"pk{j}_{ct}", bufs=3)
                    for ci in range(CT):
                        nc.tensor.matmul(ps[:], wa(ci, slice(ct * 128, ct * 128 + 128)),
                                         xkv_r[:, ci, sl],
                                         start=(ci == 0), stop=(ci == CT - 1))
                    nc.scalar.activation(k_r[:, ct, sl], ps[:], AF.Identity)

            def vtproj(mi):  # v'^T tile [keys, 256 ch + 1 t-bias col]
                ps = pp.tile([128, CV], F32, tag="st", name=f"pv{mi}", bufs=3)
                for ci in range(CT):
                    nc.tensor.matmul(ps[:], xkv_r[:, ci, mi * 128:(mi + 1) * 128],
                                     wfu(ci),
                                     start=(ci == 0), stop=(ci == CT - 1))
                nc.vector.tensor_add(vt_r[:, mi, :], ps[:], bv_s[:])

            for c in range(4):
                kproj(2 * c)
                kproj(2 * c + 1)
                for mi in range(8 * c, 8 * c + 8):
                    vtproj(mi)

            # ---- attention; chunk j's FFN runs during chunk j+1's m-loop ----
            ffn_state = {}

            def ffn_stages(j, last=False):
                """Yield (mi_trigger, emit_fn) pieces for chunk j's FFN."""
                sl = slice(j * NCH, (j + 1) * NCH)
                # Tail chunk: reuse the dead av0/av1 bank slots so the
                # out-matmul pair can overlap without extra PSUM banks.
                if last:
                    ftags = {"rbp": ("ffn", 1), "op0": ("av0", 2),
                             "op1": ("av1", 2)}
                else:
                    ftags = {k: ("ffn", 1) for k in ("rbp", "op0", "op1")}
                st_ = {}

                def s_recip():
                    av0, av1, smp = ffn_state.pop(j)
                    st_["av"] = (av0, av1)
                    r = wpool.tile([1, NCH], CDT, tag="recip", name=f"recip{j}",
                                   bufs=1)
                    with nc.allow_low_precision(reason="f32r reciprocal ~2^-13"):
                        nc.vector.reciprocal(r[:], smp[:])
                    st_["recip"] = r

                def s_rbp():
                    t, b = ftags["rbp"]
                    rbp = pp.tile([128, NCH], F32, tag=t, name=f"rbp{j}", bufs=b)
                    nc.tensor.matmul(rbp[:], onesrow[:], st_["recip"][:],
                                     start=True, stop=True)
                    st_["rbp"] = rbp
                    if last:
                        warm2 = pp.tile([128, NCH], F32, tag="st", name="warm2",
                                        bufs=3)
                        for _ in range(10):
                            nc.tensor.matmul(warm2[:], onesrow[:],
                                             st_["recip"][:], start=True,
                                             stop=True)
                        wrd2 = wpool.tile([1, 1], F32, tag="wrd", bufs=1)
                        nc.vector.tensor_copy(wrd2[:], warm2[0:1, 0:1])

                def s_avn():
                    rb = wpool.tile([128, NCH], F32, tag="rb", name=f"rb{j}", bufs=1)
                    nc.vector.tensor_copy(rb[:], st_["rbp"][:])
                    avn = wpool.tile([128, CT, NCH], CDT, tag="avn", name=f"avn{j}",
                                     bufs=1)
                    av0, av1 = st_["av"]
                    nc.vector.tensor_mul(avn[:, 0, :], av0[:], rb[:])
                    nc.vector.tensor_mul(avn[:, 1, :], av1[:], rb[:])
                    st_["avn"] = avn
                    st_["hid"] = wpool.tile([128, CT, NCH], BF16, tag="hid",
                                            name=f"hid{j}", bufs=1)
                    st_["outp"] = wpool.tile([128, CT, NCH], F32, tag="outp",
                                             name=f"outp{j}", bufs=1)

                def s_relu(ot):
                    def go():
                        nc.scalar.activation(st_["hid"][:, ot, :],
                                             st_["avn"][:, ot, :], AF.Relu,
                                             bias=b1sl(ot))
                    return go

                def s_out(ot):
                    def go():
                        t, b = ftags[f"op{ot}"]
                        op = pp.tile([128, NCH], F32, tag=t,
                                     name=f"op{j}_{ot}", bufs=b)
                        for ci in range(CT):
                            nc.tensor.matmul(
                                op[:], w2sl(ci, slice(ot * 128, ot * 128 + 128)),
                                st_["hid"][:, ci, :], start=(ci == 0), stop=(ci == CT - 1))
                        if ot == 0:
                            nc.scalar.activation(st_["outp"][:, ot, :], op[:],
                                                 AF.Identity, bias=b2sl(ot))
                        else:
                            nc.vector.tensor_scalar_add(st_["outp"][:, ot, :],
                                                        op[:], bf32[:, ot, 1:2])
                    return go

                def s_dma(ot):
                    def go():
                        nc.sync.dma_start(out_d[:, ot, sl], st_["outp"][:, ot, :])
                    return go

                return [(1, s_recip), (5, s_rbp), (7, s_avn),
                        (9, s_relu(0)), (11, s_relu(1)),
                        (14, s_out(0)), (16, s_dma(0)),
                        (17, s_out(1)), (20, s_dma(1))]

            for j in range(NJ):
                sl = slice(j * NCH, (j + 1) * NCH)
                av0 = pp.tile([128, NCH], F32, tag="av0", name=f"av0_{j}", bufs=2)
                av1 = pp.tile([128, NCH], F32, tag="av1", name=f"av1_{j}", bufs=2)
                acc = wpool.tile([128, 4, NCH], BF16, tag="acc", name=f"acc{j}",
                                 bufs=2)
                pending = ffn_stages(j - 1) if j > 0 else []

                def sp_mm(mi):
                    sp = pp.tile([128, NCH], F32, tag="st", name=f"sp{j}_{mi}",
                                 bufs=3)
                    for ci in range(CT):
                        nc.tensor.matmul(sp[:], k_r[:, ci, mi * 128:(mi + 1) * 128],
                                         xq_r[:, ci, sl], start=(ci == 0),
                                         stop=(ci == CT - 1))
                    return sp

                def exp_mm(mi, sp):
                    et = epool.tile([128, NCH], BF16, tag="et", name=f"et{j}_{mi}")
                    nc.scalar.activation(et[:], sp[:], AF.Exp,
                                         bias=vt_r[:, mi, C:C + 1])
                    return et

                # 2-deep software pipeline: scores for mi+2 and exp for mi+1
                # are emitted (= prioritized) ahead of mi's AV matmuls, so the
                # PE never waits on the PSUM->exp->SBUF round trip.
                sps = {0: sp_mm(0), 1: sp_mm(1)}
                ets = {0: exp_mm(0, sps.pop(0))}
                for mi in range(MT):
                    if mi + 2 < MT:
                        sps[mi + 2] = sp_mm(mi + 2)
                    if mi + 1 < MT:
                        ets[mi + 1] = exp_mm(mi + 1, sps.pop(mi + 1))
                    et = ets.pop(mi)
                    first, last = mi == 0, mi == MT - 1
                    nc.tensor.matmul(av0[:], vt_r[:, mi, 0:128], et[:], start=first, stop=last)
                    nc.tensor.matmul(av1[:], vt_r[:, mi, 128:256], et[:], start=first, stop=last)
                    if j == NJ - 1 and mi == MT - 1:
                        last_et = et  # summed by a direct matmul, not the chain
                    else:
                        g = mi // 8
                        if mi % 8 == 0:
                            nc.vector.tensor_copy(acc[:, g, :], et[:])
                        else:
                            nc.vector.tensor_add(acc[:, g, :], acc[:, g, :], et[:])
                    while pending and pending[0][0] == mi:
                        pending.pop(0)[1]()
                # softmax denominators: accumulating partition-reduce matmuls
                lastj = j == NJ - 1
                smp = pp.tile([1, NCH], F32, tag="st" if lastj else "ffn",
                              name=f"smp{j}", bufs=3 if lastj else 1)
                for g in range(4):
                    nc.tensor.matmul(smp[:], ones_b[:], acc[:, g, :],
                                     start=(g == 0), stop=(g == 3 and not lastj))
                if lastj:
                    nc.tensor.matmul(smp[:], ones_b[:], last_et[:],
                                     start=False, stop=True)
                    warm = pp.tile([1, NCH], F32, tag="st", name="warm", bufs=3)
                    for _ in range(14):
                        nc.tensor.matmul(warm[:], ones_b[:], last_et[:],
                                         start=True, stop=True)
                    wrd = wpool.tile([1, 1], F32, tag="wrd", bufs=1)
                    nc.vector.tensor_copy(wrd[:], warm[:, 0:1])
                ffn_state[j] = (av0, av1, smp)
            for _, fn in ffn_stages(NJ - 1, last=True):
                fn()
    nc.compile()
    return nc


_NC_CACHE = None


def _get_nc():
    global _NC_CACHE
    if _NC_CACHE is None:
        _NC_CACHE = _build()
    return _NC_CACHE


def _fold(a, dt=np.float32):
    """[C, X] -> [128, CT, X] with channel tile as middle dim, contiguous."""
    x = np.asarray(a, dtype=dt)
    return np.ascontiguousarray(x.reshape(CT, 128, -1).transpose(1, 0, 2))


def _make_in_maps(inputs):
    import ml_dtypes

    bf16 = ml_dtypes.bfloat16
    f = {k: np.asarray(v, np.float64) for k, v in inputs.items()}
    query_input = f["query_input"].reshape(B, C, N)
    key_value_input = f["key_value_input"].reshape(B, C, N)

    # Host-side algebra (see module docstring): softmax-invariant rewrite of
    # the score bilinear form, and the W1-fold through the value projection.
    A = f["Wk"].T @ f["Wq"]                      # [Cin, Cin]
    u = f["Wk"].T @ f["bq"]                      # [Cin]
    Fu = np.concatenate([(f["W1"] @ f["Wv"]).T, u[:, None]], axis=1)  # [C, CV]
    wpack = np.concatenate(
        [A, Fu, f["W2"].T, f["b1"][:, None], f["b2"][:, None]], axis=1)
    bvp = np.concatenate([f["W1"] @ f["bv"], [0.0]])                  # [CV]

    base = {
        "wpack": _fold(wpack, bf16),
        "bvb": np.ascontiguousarray(
            np.broadcast_to(bvp.astype(np.float32)[None, :], (128, CV))),
    }
    in_maps = []
    for core in range(8):
        b, h = divmod(core, 2)
        m = dict(base)
        m["xq"] = _fold(query_input[b][:, h * NL:(h + 1) * NL], bf16)
        m["xkv"] = _fold(key_value_input[b], bf16)
        in_maps.append(m)
    return in_maps


def kernel(query_input, key_value_input, Wq, bq, Wk, bk, Wv, bv, W1, b1, W2, b2):
    in_maps = _make_in_maps(dict(
        query_input=query_input, key_value_input=key_value_input,
        Wq=Wq, bq=bq, Wk=Wk, bk=bk, Wv=Wv, bv=bv, W1=W1, b1=b1, W2=W2, b2=b2))
    nc = _get_nc()
    res = run_bass_kernel_spmd(nc, in_maps, core_ids=list(range(8)))

    out = np.empty((B, C, N), dtype=np.float32)
    for core in range(8):
        b, h = divmod(core, 2)
        o = res.results[core]["out"]  # [128, CT, NL]
        out[b][:, h * NL:(h + 1) * NL] = o.transpose(1, 0, 2).reshape(C, NL)
    return out.reshape(B, C, H, W)

